# revision 18
# baseline (speedup 1.0000x reference)
"""Differentiable random-forest layer (inference path) on 8 Trainium2 cores.

Computation (per reference):
    d     = sigmoid(einsum('bf,tfn->btn', x, W))        # [B, T, 255]
    route = prod_l where(IS_LEFT, d[..n..], 1-d[..n..]) # [B, T, 256]
    out   = clip(einsum('btl,tlc->bc', route, P) / T, 0, 1)

Shapes: B=4096, F=1024, T=10 trees, 255 nodes / 256 leaves, C=1000.

Sharding: data-parallel over batch. Each of the 8 cores handles 512 rows;
no collectives are needed (weights/probs are broadcast to every core).

Both matmuls run as fp8e4m3 DoubleRow (256-deep contraction per PE pass):
  mm1   : logits[b,512] += x8[k-pair, b].T @ (16 W8)[k-pair, tree-pair]
  sig   : d = sigmoid(logits / 16)          (ACT, psum->sbuf bf16, d only)
  route : R_{l+1} = [R_l*d_l, R_l - R_l*d_l]  (DVE; 16x scale folded at l=0;
          the last layer writes fp8 with the two leaf-halves byte-interleaved)
  transp: ONE XBAR DMA transpose of the packed fp8 pairs viewed as uint16
  mm2   : psum[b,c] += routeT8.T @ dP8  (DoubleRowSwInterleave: the packed
          pair layout is exactly the interleaved+column-reversed weight
          storage this mode expects; the host reverses each 128-batch block
          of x's columns so mm2's outputs land in natural batch order)
  store : psum (f32) -> DRAM (ACT copies psum->sbuf, scale applied on host)

Precision: P is mean-centered on the host (P = Pbar + dP); the device only
computes route . dP in fp8, and the constant sum_t Pbar term (route sums to
1 per tree) plus the 2^-25 scale are applied on the host during unsharding.
Sim-predicted rel err ~1.1e-2 vs the 2e-2 gate (measured 1.06e-2 on HW).

The routing uses the "concat" (decision-bit-as-LSB) leaf ordering so every
DVE read/write is contiguous; the host pre-permutes W's node axis (per-layer
bit-reversal) and P's leaf axis (8-bit reversal) to compensate, which is free.
"""

from contextlib import ExitStack

import numpy as np
import ml_dtypes

import concourse.bass as bass
import concourse.bacc as bacc
import concourse.mybir as mybir
import concourse.tile as tile
from concourse.bass_utils import run_bass_kernel_spmd

N_CORES = 8
B, F, T, NODES, LEAFS, C = 4096, 1024, 10, 255, 256, 1000
B_LOC = B // N_CORES            # 512 batch rows per core
BCH = B_LOC // 128              # 4 batch chunks of 128
KF = F // 128                   # 8 contraction chunks for mm1
KP = KF // 2                    # 4 fp8 DoubleRow k-pairs
TP = T // 2                     # 5 tree-pairs (2 trees -> 512 psum cols)
NPAD = 256                      # per-tree node columns, padded 255 -> 256
N_LAYERS = 8

W_SCALE = 16.0                  # W * 16 in fp8; sigmoid applies 1/16
R_SCALE = 16.0                  # route * 16 in fp8 (folded into layer 0)
P_SCALE = float(2 ** 21)        # centered dP * 2^21 in fp8
OUT_SCALE = 1.0 / (R_SCALE * P_SCALE)   # host-side psum -> output scale

BF16 = mybir.dt.bfloat16
FP8 = mybir.dt.float8e4
U16 = mybir.dt.uint16
F32 = mybir.dt.float32
Sigmoid = mybir.ActivationFunctionType.Sigmoid
Copy = mybir.ActivationFunctionType.Copy
DR = mybir.MatmulPerfMode.DoubleRow
DRSW = mybir.MatmulPerfMode.DoubleRowSwInterleave


def _bitrev(x: int, bits: int) -> int:
    r = 0
    for _ in range(bits):
        r = (r << 1) | (x & 1)
        x >>= 1
    return r


# Node-axis permutation: d'[.., off+q] = d[.., off+bitrev_l(q)] per layer l
NODE_PERM = np.empty(NODES, dtype=np.int64)
for _l in range(N_LAYERS):
    _off = (1 << _l) - 1
    for _q in range(1 << _l):
        NODE_PERM[_off + _q] = _off + _bitrev(_q, _l)
# Leaf-axis permutation: P'[t, q, :] = P[t, bitrev_8(q), :]
LEAF_PERM = np.array([_bitrev(q, N_LAYERS) for q in range(LEAFS)], dtype=np.int64)


def build_program() -> bass.Bass:
    nc = bacc.Bacc()

    # x: per-chunk contiguous [part(k_lo), chunk, k_hi, col]; cols reversed
    xT = nc.dram_tensor("xT", [128, BCH, KF, 128], FP8, kind="ExternalInput")
    # W is j-major: one contiguous block per tree-pair j covering all KF chunks
    w = nc.dram_tensor("w", [TP, 128, KF * 2 * NPAD], FP8, kind="ExternalInput")
    p = nc.dram_tensor("p", [128, 2 * T * C], FP8, kind="ExternalInput")
    out = nc.dram_tensor("out", [B_LOC, C], F32, kind="ExternalOutput")

    with tile.TileContext(nc) as tc, ExitStack() as ctx:
        resident = ctx.enter_context(tc.tile_pool(name="resident", bufs=1))
        x_all = resident.tile([128, BCH, KF, 128], FP8, tag="x_all", name="x_all")
        w_all = resident.tile([128, TP, KF, 2 * NPAD], FP8, tag="w_all", name="w_all")
        p_all = resident.tile([128, 2, T * C], FP8, tag="p_all", name="p_all")
        # Outstanding DMAs on a ring round-robin their packets, so a large
        # low-priority transfer delays the completion of everything sharing
        # the ring. The x/W stream (the mm1 critical path) therefore gets the
        # sync ring to itself in consumption order; P (2.5MB, first needed by
        # mm2 ~15us later) is dispatched from the ACT queue, positioned in
        # the emission stream after chunk 1's first sigmoid so its packets
        # only enter HBM once the W stream is essentially done.
        nc.sync.dma_start(x_all[:, 0], xT[:, 0])
        nc.sync.dma_start(w_all[:, 0, :, :], w[0])
        nc.sync.dma_start(w_all[:, 1, :, :], w[1])

        dpool = ctx.enter_context(tc.tile_pool(name="dps", bufs=1, space="PSUM"))
        opool = ctx.enter_context(tc.tile_pool(name="ops", bufs=3, space="PSUM"))
        wpool = ctx.enter_context(tc.tile_pool(name="wps", bufs=1, space="PSUM"))
        work = ctx.enter_context(tc.tile_pool(name="work", bufs=2))

        # ---- PE warmup: the first ~9us are preamble+DMA-bound; run dummy
        # matmuls so the HAM clock gate is (mostly) at full speed when real
        # work lands. ----
        warm_in = work.tile([128, 128], BF16, tag="warm", name="warm_in", bufs=1)
        nc.vector.memset(warm_in[:, :], 0.0)
        warm_ps = wpool.tile([128, 128], F32, tag="warm", name="warm_ps", bufs=1)
        for _ in range(32):
            nc.tensor.matmul(warm_ps[:, :], warm_in[:, :], warm_in[:, :])

        def anchor_dma(dst_tile, anchor_src, dma_out, dma_in):
            # Pace a bulk load: outstanding DMAs round-robin ring bandwidth,
            # so a 1-element write into the destination (dependent on an
            # earlier pipeline event) keeps this DMA out of the ring until
            # the anchor is reached.
            nc.vector.tensor_copy(dst_tile, anchor_src)
            nc.sync.dma_start(dma_out, dma_in)

        def emit_mm2(rT8, bsl, nchunks=((0, 512), (512, C - 512))):
            # mm2: psum[b, c] += routeT8.T @ dP8 (SwInterleave DoubleRow over
            # the packed leaf pairs), accumulated over trees. Host applies the
            # 2^-25 scale and adds the constant sum_t Pbar term; ACT copies
            # the f32 PSUM to SBUF and dispatches the store.
            osb = work.tile([128, C], F32, tag="osb", name="osb")
            for n0, nsz in nchunks:
                ops = opool.tile([128, 512], F32, tag="ops", name="ops")
                for t_ in range(T):
                    nc.tensor.matmul(
                        ops[:, 0:nsz],
                        rT8[:, t_, :].rearrange("p (i m) -> p i m", i=2),
                        p_all[:, :, t_ * C + n0 : t_ * C + n0 + nsz],
                        start=(t_ == 0),
                        stop=(t_ == T - 1),
                        perf_mode=DRSW,
                    )
                nc.scalar.activation(osb[:, n0 : n0 + nsz], ops[:, 0:nsz], Copy)
                nc.scalar.dma_start(out[bsl, n0 : n0 + nsz], osb[:, n0 : n0 + nsz])

        def emit_mm1_j(bi, j, ddb):
            # d logits for tree-pair j of chunk bi (DoubleRow over k-pairs),
            # then a single sigmoid (dbar is derived during routing).
            dps = dpool.tile([128, 2, NPAD], F32, tag="dps", name="dps", bufs=3)
            for m in range(KP):
                nc.tensor.matmul(
                    dps[:, :, :],
                    x_all[:, bi, 2 * m : 2 * m + 2, :],
                    w_all[:, j, 2 * m : 2 * m + 2, :],
                    start=(m == 0),
                    stop=(m == KP - 1),
                    perf_mode=DR,
                )
            # d = sigmoid(logits / 16), psum -> sbuf bf16 (255 valid cols/tree)
            nc.scalar.activation(
                ddb[:, 2 * j : 2 * j + 2, 0:NODES],
                dps[:, :, 0:NODES],
                Sigmoid,
                scale=1.0 / W_SCALE,
            )

        def emit_routing(ddb):
            # ---- routing: hierarchical doubling, concat ordering ----
            # R_{l+1}[0:w]  = R_l[0:w] * d_l   (decision bit 0 -> left)
            # R_{l+1}[w:2w] = R_l[0:w] - R_{l+1}[0:w]      (= R_l * (1-d_l))
            # The fp8 route scale (x16) is folded into layer 0. The last
            # layer writes fp8 with the leaf-halves byte-interleaved; one
            # uint16-viewed XBAR transpose then yields mm2's SwInterleave
            # stationary layout directly.
            Ra = work.tile([128, T, LEAFS], BF16, tag="Ra", name="Ra")
            Rb = work.tile([128, T, LEAFS], BF16, tag="Rb", name="Rb")
            routeC8 = work.tile([128, T, 128, 2], FP8, tag="routeC8", name="routeC8")
            d0 = ddb[:, :, 0:1]
            nc.vector.tensor_scalar_mul(Ra[:, :, 0:1], d0, R_SCALE)
            nc.vector.tensor_scalar(
                Ra[:, :, 1:2], d0, -R_SCALE, R_SCALE,
                mybir.AluOpType.mult, mybir.AluOpType.add,
            )
            cur, nxt = Ra, Rb
            for l in range(1, N_LAYERS - 1):
                w_l = 1 << l          # prefixes at layer l
                off = w_l - 1         # first node index of layer l
                lo, hi = nxt[:, :, 0:w_l], nxt[:, :, w_l : 2 * w_l]
                nc.vector.tensor_mul(lo, cur[:, :, 0:w_l], ddb[:, :, off : off + w_l])
                nc.vector.tensor_sub(hi, cur[:, :, 0:w_l], lo)
                cur, nxt = nxt, cur
            # last layer: fp8 output runs at ~half DVE rate, so split the
            # trees across DVE and GPSIMD to shorten the chain
            off, w_l, GS = 127, 128, 6
            lo, hi = routeC8[:, :, :, 0], routeC8[:, :, :, 1]
            nc.vector.tensor_mul(
                lo[:, 0:GS], cur[:, 0:GS, 0:w_l], ddb[:, 0:GS, off : off + w_l]
            )
            nc.gpsimd.tensor_mul(
                lo[:, GS:], cur[:, GS:, 0:w_l], ddb[:, GS:, off : off + w_l]
            )
            nc.vector.tensor_sub(hi[:, 0:GS], cur[:, 0:GS, 0:w_l], lo[:, 0:GS])
            nc.gpsimd.tensor_sub(hi[:, GS:], cur[:, GS:, 0:w_l], lo[:, GS:])
            # one 2-byte XBAR transpose of the packed pairs: [b, pair] ->
            # [leaf_low, t, b-pairs] = SwInterleave stationary storage
            rT8 = work.tile([128, T, 256], FP8, tag="rT8", name="rT8", bufs=3)
            nc.sync.dma_start_transpose(
                rT8[:, :, :].bitcast(U16), routeC8[:, :, :, :].bitcast(U16)
            )
            return rT8

        # Emission order = desired per-engine instruction order. mm1 runs
        # chunk-SEQUENTIAL so ddb0 completes as early as possible and the
        # four routing/transpose chains (DVE + XBAR) pipeline ahead of their
        # mm2 consumers with several microseconds of slack each.
        ddb0 = work.tile([128, T, NPAD], BF16, tag="ddb", name="ddb0", bufs=4)
        ddb1 = work.tile([128, T, NPAD], BF16, tag="ddb", name="ddb1", bufs=4)
        ddb2 = work.tile([128, T, NPAD], BF16, tag="ddb", name="ddb2", bufs=4)
        ddb3 = work.tile([128, T, NPAD], BF16, tag="ddb", name="ddb3", bufs=4)
        # W/x loads are paced with dependency anchors (see anchor_dma) so the
        # HBM ring always serves the next-needed block at full bandwidth:
        # w1 enters once x0 has landed; w2..w4/x1..x3 enter as chunk-0/1
        # sigmoids confirm the pipeline has consumed the earlier blocks.
        anchor_dma(w_all[0:1, 2, 0:1, 0:1], x_all[0:1, 0, 0:1, 0:1],
                   w_all[:, 2, :, :], w[2])
        emit_mm1_j(0, 0, ddb0)
        anchor_dma(w_all[0:1, 3, 0:1, 0:1], ddb0[0:1, 0:1, 0:1],
                   w_all[:, 3, :, :], w[3])
        anchor_dma(x_all[0:1, 1, 0:1, 0:1], ddb0[0:1, 0:1, 0:1],
                   x_all[:, 1], xT[:, 1])
        emit_mm1_j(0, 1, ddb0)
        anchor_dma(w_all[0:1, 4, 0:1, 0:1], ddb0[0:1, 2:3, 0:1],
                   w_all[:, 4, :, :], w[4])
        emit_mm1_j(0, 2, ddb0)
        anchor_dma(x_all[0:1, 2, 0:1, 0:1], ddb0[0:1, 4:5, 0:1],
                   x_all[:, 2], xT[:, 2])
        emit_mm1_j(0, 3, ddb0)
        emit_mm1_j(0, 4, ddb0)
        emit_mm1_j(1, 0, ddb1)
        # Delay P's entry into the HBM ring until the W stream is essentially
        # done: the tile scheduler hoists dependency-free DMAs to the front,
        # so anchor the P load behind chunk 1's first sigmoid via a 1-element
        # write into its destination buffer (WAR forces the DMA after it).
        nc.vector.tensor_copy(p_all[0:1, 0:1, 0:1], ddb1[0:1, 0:1, 0:1])
        nc.scalar.dma_start(p_all[:, :, :], p.rearrange("p (k n) -> p k n", k=2))
        emit_mm1_j(1, 1, ddb1)
        anchor_dma(x_all[0:1, 3, 0:1, 0:1], ddb1[0:1, 2:3, 0:1],
                   x_all[:, 3], xT[:, 3])
        for j in range(2, TP):
            emit_mm1_j(1, j, ddb1)
        rT0 = emit_routing(ddb0)
        for j in range(TP):
            emit_mm1_j(2, j, ddb2)
        rT1 = emit_routing(ddb1)
        for j in range(TP):
            emit_mm1_j(3, j, ddb3)
        rT2 = emit_routing(ddb2)
        emit_mm2(rT0, bass.ts(0, 128))
        rT3 = emit_routing(ddb3)
        emit_mm2(rT1, bass.ts(1, 128))
        emit_mm2(rT2, bass.ts(2, 128))
        # final chunk: finer output blocks so the last copy+store tail is short
        emit_mm2(rT3, bass.ts(3, 128), nchunks=((0, 512), (512, 256), (768, 128), (896, C - 896)))

    nc.finalize()
    return nc


_CACHED_NC = None
_WARMED = False


def _get_nc() -> bass.Bass:
    global _CACHED_NC
    if _CACHED_NC is None:
        _CACHED_NC = build_program()
    return _CACHED_NC


def _prep_inputs(l_input, cnn_w, final_probabilities):
    e4 = ml_dtypes.float8_e4m3
    x = np.ascontiguousarray(np.asarray(l_input, dtype=np.float32))
    W = np.asarray(cnn_w, dtype=np.float32)[:, :, NODE_PERM]
    P = np.asarray(final_probabilities, dtype=np.float32)[:, LEAF_PERM, :] * (1.0 / T)

    # mean-center P over leaves; the constant term is added on the host
    Pm = P.mean(axis=1)                    # [T, C]
    dP = P - Pm[:, None, :]                # [T, LEAFS, C]
    base = Pm.sum(axis=0).astype(np.float32)   # [C]

    # x [B, F] -> [128(k_lo), B/128 chunks, KF, 128 cols] fp8, with each
    # 128-batch block's columns REVERSED (SwInterleave emits outputs in
    # reverse column order; this pre-reversal makes mm2 outputs land in
    # natural batch order)
    xr = (
        x.reshape(B // 128, 128, KF, 128)[:, ::-1]   # reverse cols per chunk
        .transpose(3, 0, 2, 1)                       # [k_lo, chunk, KF, col]
        .astype(e4)
    )
    xr = np.ascontiguousarray(xr)
    # W [T, F, N] x16 -> pad nodes to 256 -> [F, T, 256] -> j-major
    Wp = np.zeros((T, F, NPAD), dtype=np.float32)
    Wp[:, :, :NODES] = W * W_SCALE
    Wr = (
        np.ascontiguousarray(Wp.transpose(1, 0, 2))
        .astype(e4)
        .reshape(KF, 128, TP, 2 * NPAD)
        .transpose(2, 1, 0, 3)
        .reshape(TP, 128, KF * 2 * NPAD)
    )
    Wr = np.ascontiguousarray(Wr)
    # dP [T, 256, C] * 2^21 -> [128(leaf_low), leaf-chunk, T*C] fp8
    Pr = np.ascontiguousarray(
        (dP * P_SCALE).reshape(T, 2, 128, C).transpose(2, 1, 0, 3)
    ).astype(e4).reshape(128, 2 * T * C)
    return xr, Wr, Pr, base


def _run(inputs, trace=False, trace_cores=None):
    xr, Wr, Pr, base = _prep_inputs(
        inputs["l_input"], inputs["cnn_w"], inputs["final_probabilities"]
    )
    in_maps = [
        {
            "xT": np.ascontiguousarray(xr[:, c * BCH : (c + 1) * BCH]),
            "w": Wr,
            "p": Pr,
        }
        for c in range(N_CORES)
    ]
    global _WARMED
    if not _WARMED and not trace:
        # one discarded execution to warm the device path (DMA rings, NEFF
        # residency, clock state) so the measured run is at steady state
        try:
            run_bass_kernel_spmd(
                _get_nc(), in_maps, core_ids=list(range(N_CORES)), trace=False
            )
        except Exception:
            pass
        _WARMED = True
    last_err = None
    for attempt in range(3):
        try:
            res = run_bass_kernel_spmd(
                _get_nc(),
                in_maps,
                core_ids=list(range(N_CORES)),
                trace=trace,
                trace_cores=trace_cores,
            )
            break
        except Exception as e:  # transient NRT device errors: retry
            last_err = e
            if attempt == 2:
                raise
            import time as _time

            _time.sleep(5)
    out = np.concatenate([res.results[c]["out"] for c in range(N_CORES)], axis=0)
    out = out * OUT_SCALE + base[None, :]
    return out, res


def kernel(**inputs) -> np.ndarray:
    out, _ = _run(inputs)
    return out
